# revision 20
# baseline (speedup 1.0000x reference)
"""Trainium2 Bass kernel for causal attention with relative-position bias.

Problem (hardcoded): B=16 heads, S=2048, Dh=64, fp32 I/O.
  dots = Q@K^T; bias pos=Q@R_w^T+R_b gathered by sign(j-i)+1; causal mask
  (-1e10 above diag); softmax(dots/sqrt(512)); out = probs@V.

Algebra: within row q the gathered bias is a constant pos0[q] for k<q and
pos1[q] at k==q (k>q masked). Softmax is invariant to per-row constants, so
only the diagonal needs the delta v[q] = Q[q].(R_w[1]-R_w[0]) + R_b[1]-R_b[0].
Logits are small (|z|<=~2.2) so exp runs without max subtraction.

Layout: scores computed transposed, S^T[k,q] (k on partitions):
  S^T = (K^T chunk).T @ Q^T      (lhsT=K^T[64,128], rhs=Q^T[64,ncols])
  out^T[d,q]+denominator row = [V|1].T @ exp(S^T)  (accumulated over chunks)

Diagonal tiles: one accumulate-matmul  A_ui^T @ dcomb_ki  is emitted before
the QK matmul, where A_ui[m,k]=[m<=k] (so the product is a column cumsum) and
dcomb[m,q] = v[q]*([m==q]-[m==q+1]) + (-V0*[m>q] + rbd*[m==q]).  The cumsum
turns this into v[q]*[k==q] - V0*(k-q)*[k>q] + rbd*[k>=q]: position-bias
delta on the diagonal, -V0*(k-q) above it (exp == 0 in fp16), nothing below.
The exp is then uniform and PV needs no diagonal special-casing.

Q^T/K^T: head 0 via PE-mode transposes at startup (PE idle, warms HAM),
head 1 via xbar-DMA transposes overlapped with head 0's main loop.  The
transposed strips are [128, S]: rows 0:64 hold K^T/Q^T, rows 64:128 a
gpsimd SBUF-to-SBUF duplicate, so the two 512-col QK chunks of a fill run
CONCURRENTLY in independent PE row groups (contraction is only 64) --
auto row-tiling from the operands' base partition.

Input loads ride the sync HWDGE ring so the scalar (ACT) engine queue
carries only the exp stream -- exp at 1 elem/lane/cycle @1.2GHz over the
causal region is the binding resource.

Sharding: 16 heads -> 8 NeuronCores, 2 heads/core, no communication.
"""

import os
import sys

if "/opt/trn_rl_repo" not in sys.path:
    sys.path.insert(0, "/opt/trn_rl_repo")

import numpy as np

import concourse.bacc as bacc
import concourse.mybir as mybir
import concourse.tile as tile
from concourse.bass_utils import run_bass_kernel_spmd
from concourse.masks import make_identity, make_lower_triangular, make_upper_triangular

B, S, DH = 16, 2048, 64
N_CORES = 8
HPC = B // N_CORES  # heads per core
P = 128
NT = S // P  # 16 q/k tiles per head
HT = NT // 2  # tiles per load half
VW = 66  # V row width in SBUF: 64 values + ones col + pad (66*2B keeps 4B align)
OW = 80  # out^T rows padded to xbar multiple of 16 (64 vals + denom + 15 pad)
PH = 1024  # q-phase width
INV_SCALE = float(1.0 / np.sqrt(np.float32(512.0)))
V0 = 1000.0  # per-step causal mask magnitude; exp((z-V0)/scale) == 0 in fp16

f16 = mybir.dt.float16
f32 = mybir.dt.float32


def _emit(ctx, tc, q_d, k_d, v_d, rw_d, rb_d, out_d):
    nc = tc.nc
    AF = mybir.ActivationFunctionType

    const = ctx.enter_context(tc.tile_pool(name="const", bufs=1))
    ld = ctx.enter_context(tc.tile_pool(name="ld", bufs=2))
    hp = ctx.enter_context(tc.tile_pool(name="hp", bufs=2))
    slabp = ctx.enter_context(tc.tile_pool(name="slab", bufs=4))
    outp = ctx.enter_context(tc.tile_pool(name="outp", bufs=2))
    psc = ctx.enter_context(tc.tile_pool(name="psc", bufs=3, space="PSUM"))
    pout = ctx.enter_context(tc.tile_pool(name="pout", bufs=1, space="PSUM"))

    # PE warm-up junk buffer; memset first so the warm matmuls start early
    junk = const.tile([P, 512], f16)
    nc.gpsimd.memset(junk[:], 0.0)

    # broadcast R_w rows 0+1 and R_b[0:2] to all partitions (0-step DMA reads)
    # rbc rides the scalar ring: on the sync ring its tiny broadcast packets
    # finish ~5us late (round-robined behind the 12 big input loads), and the
    # whole DVE const chain -> dcomb -> first exp waits on it.
    rbc = const.tile([P, 2 * DH + 2], f32)
    nc.scalar.dma_start(
        out=rbc[:, 0 : 2 * DH], in_=rw_d[0:2, :].flatten()[None, :].partition_broadcast(P)
    )
    nc.scalar.dma_start(
        out=rbc[:, 2 * DH : 2 * DH + 2], in_=rb_d[None, 0:2].partition_broadcast(P)
    )

    # constants ----------------------------------------------------------
    idm = const.tile([P, P], f16)  # fp16 identity for PE-mode transpose
    make_identity(nc, idm[:])
    aui = const.tile([P, P], f16)  # A_ui[m,k] = 1 for m<=k (cumsum matmul lhsT)
    make_upper_triangular(nc, aui[:], val=1.0, diag=True)
    # idp[m,j] = [m==j] - [m==j+1]: cumsum of idp*v gives diag(v)
    idp = const.tile([P, P], f16)
    make_identity(nc, idp[:])
    nc.gpsimd.affine_select(
        out=idp[:], in_=idp[:], compare_op=mybir.AluOpType.not_equal,
        fill=-1.0, base=-1, pattern=[[-1, P]], channel_multiplier=1,
    )
    # bneg[m,q] = -V0*[m>q] + rbd*[m==q]; cumsum gives the causal mask + rbd
    id01 = const.tile([P, P], mybir.dt.int8)
    make_identity(nc, id01[:])
    bneg = const.tile([P, P], f16)
    make_lower_triangular(nc, bneg[:], val=-V0, diag=False)

    rd16 = const.tile([P, DH], f16)  # R_w[1]-R_w[0], fp16, bcast on partitions
    rbraw = const.tile([P, 2], f32)  # col 0: R_b[1]-R_b[0] (raw, pre-scale)
    rbf16 = const.tile([P, 1], f16)

    def emit_rb_consts():
        # Emitted AFTER the first head's casts: these wait on the rbc DMAs,
        # and the DVE queue is strict FIFO — put them where the wait is free.
        nc.vector.tensor_sub(rd16[:], rbc[:, DH : 2 * DH], rbc[:, 0:DH])
        nc.vector.tensor_sub(
            rbraw[:, 0:1], rbc[:, 2 * DH + 1 : 2 * DH + 2], rbc[:, 2 * DH : 2 * DH + 1]
        )
        nc.vector.tensor_copy(rbf16[:], rbraw[:, 0:1])
        nc.vector.copy_predicated(bneg[:], id01[:], rbf16[:, 0:1].to_broadcast([P, P]))

    # PE warm-up: junk matmuls while DMAs load (HAM at 8/8 by the first QK).
    warm0 = psc.tile([P, PH], f32, tag="sc")
    for _ in range(6):
        nc.tensor.matmul(
            warm0[:, 0:512], lhsT=junk[:, 0:P], rhs=junk[:], start=True,
            stop=True, skip_group_check=True,
        )

    def junk_into(count=1, n=P, t=None):
        # junk matmuls keep the HAM clock gate at 8/8 across micro-gaps.
        # Targets are PSUM tiles whose data is about to be overwritten
        # (start=True only resets has_written bits; readers never see junk).
        if t is None:
            t = warm0
        for _ in range(count):
            nc.tensor.matmul(
                t[0:32, 0:n], lhsT=junk[:, 0:32], rhs=junk[:, 0:n], start=True,
                stop=True, skip_group_check=True,
            )

    # preload the exp table set at t=0 so ACT_TABLE_LOAD overlaps input DMA
    tl16 = const.tile([P, 8], f16)
    nc.scalar.activation(tl16[:], junk[:, 0:8], AF.Exp, scale=0.0)

    def issue_loads(h):
        # fp32 loads on the scalar HWDGE ring, halves interleaved k0,q0,v0,
        # k1,q1,v1 so phase-0 operands land first.
        q32 = ld.tile([P, NT * DH], f32, tag="q32")
        k32 = ld.tile([P, NT * DH], f32, tag="k32")
        v32 = ld.tile([P, NT * DH], f32, tag="v32")
        for src, dst in ((k_d, k32), (q_d, q32), (v_d, v32)):
            for hf in range(2):
                tsl = slice(hf * HT, (hf + 1) * HT)
                nc.sync.dma_start(
                    out=dst[:].rearrange("p (n d) -> p n d", d=DH)[:, tsl, :],
                    in_=src[h].rearrange("(n p) d -> p n d", p=P)[:, tsl, :],
                )
        return q32, k32, v32

    def prep_cast(h, q32, k32, v32):
        qf = hp.tile([P, NT * DH], f16, tag="qf")
        kf = hp.tile([P, NT * DH], f16, tag="kf")
        nc.vector.tensor_copy(kf[:], k32[:])
        nc.vector.tensor_copy(qf[:], q32[:])
        return qf, kf

    def prep_dcomb(h, qf, v32):
        vaug = hp.tile([P, NT * VW], f16, tag="vaug")
        v3 = vaug[:].rearrange("p (n e) -> p n e", e=VW)
        nc.vector.tensor_copy(
            v3[:, :, 0:DH], v32[:].rearrange("p (n d) -> p n d", d=DH)
        )
        nc.gpsimd.memset(v3[:, :, DH : DH + 1], 1.0)

        # dcomb strip: per k-tile, idp*v[q] + bneg (see module docstring)
        t2 = ld.tile([P, NT * DH], f16, tag="t2")
        t2_3 = t2[:].rearrange("p (n d) -> p n d", d=DH)
        vq = hp.tile([P, NT], f32, tag="vq")
        vq16 = hp.tile([P, NT], f16, tag="vq16")
        dcomb = hp.tile([P, NT * P], f16, tag="dcomb")
        dcomb3 = dcomb[:].rearrange("p (n j) -> p n j", j=P)
        qf3 = qf[:].rearrange("p (n d) -> p n d", d=DH)
        for hf in range(2):
            sl = slice(hf * HT, (hf + 1) * HT)
            nc.vector.tensor_mul(
                t2_3[:, sl, :], qf3[:, sl, :],
                rd16[:, None, :].to_broadcast([P, HT, DH]),
            )
            nc.vector.tensor_reduce(
                out=vq[:, sl], in_=t2_3[:, sl, :],
                axis=mybir.AxisListType.X, op=mybir.AluOpType.add,
            )
            nc.vector.tensor_copy(vq16[:, sl], vq[:, sl])
            nc.vector.tensor_mul(
                dcomb3[:, sl, :],
                idp[:, None, :].to_broadcast([P, HT, P]),
                vq16[:, sl, None].to_broadcast([P, HT, P]),
            )
            nc.vector.tensor_add(
                dcomb3[:, sl, :], dcomb3[:, sl, :],
                bneg[:, None, :].to_broadcast([P, HT, P]),
            )
        return v3, dcomb3

    def prep_tp_pe(qf, kf):
        """PE-mode transposes (startup: DMAs busy, PE idle)."""
        def pe_transpose_to(src, tag):
            dst = hp.tile([P, S], f16, tag=tag, name=tag)
            tp = psc.tile([P, PH], f32, tag="sc")
            tp16 = tp[:].bitcast(f16)  # [128, 2048] fp16 view
            s3 = src[:].rearrange("p (n d) -> p n d", d=DH)
            for t in range(NT):
                nc.tensor.transpose(
                    tp16[0:DH, t * P : (t + 1) * P], s3[:, t, :], idm[:]
                )
                if t % 3 == 1:
                    junk_into()
            nc.scalar.copy(dst[0:DH, 0 : S // 2], tp16[0:DH, 0 : S // 2])
            nc.scalar.copy(dst[0:DH, S // 2 : S], tp16[0:DH, S // 2 : S])
            nc.gpsimd.dma_start(out=dst[DH:P, 0 : S // 2], in_=dst[0:DH, 0 : S // 2])
            nc.gpsimd.dma_start(out=dst[DH:P, S // 2 : S], in_=dst[0:DH, S // 2 : S])
            return dst

        kt = pe_transpose_to(kf, "kt")
        qt = pe_transpose_to(qf, "qt")
        return qt, kt

    def prep_tp_xbar(qf, kf):
        """xbar-DMA transposes (mid-flight: PE busy, DMA rings quiet).
        Folds on the sync ring, unfolds on the gpsimd (SWDGE) ring."""
        def xbar_transpose_to(src, tag):
            dst = hp.tile([P, S], f16, tag=tag, name=tag)
            fold = ld.tile([P, 8 * P], f16, tag="fold" + tag)
            nc.sync.dma_start_transpose(
                out=fold[:].rearrange("p (m r) -> p m r", r=P), in_=src[:]
            )
            d4 = dst[0:DH, :].rearrange("d (m j r) -> d m j r", j=2, r=P)
            f3 = fold[:].rearrange("p (m r) -> p m r", r=P)
            nc.gpsimd.dma_start(out=d4[:, :, 0, :], in_=f3[0:DH])
            nc.gpsimd.dma_start(out=d4[:, :, 1, :], in_=f3[DH:P])
            nc.gpsimd.dma_start(out=dst[DH:P, 0 : S // 2], in_=dst[0:DH, 0 : S // 2])
            nc.gpsimd.dma_start(out=dst[DH:P, S // 2 : S], in_=dst[0:DH, S // 2 : S])
            return dst

        kt = xbar_transpose_to(kf, "kt")
        qt = xbar_transpose_to(qf, "qt")
        return qt, kt

    def main_loop(h, qt, kt, v3, dcomb3, outTs, phases, split_last=False):
        for ph in phases:
            lo, hi = ph * PH, (ph + 1) * PH
            split = split_last and ph == phases[-1]
            fills = []
            for ki in range(NT):
                q0 = P * ki
                base = max(q0, lo)
                if base < hi:
                    fills.append((ki, q0, base, hi - base))
            outT = pout.tile([DH + 1, PH], f32, tag="outT")
            nf = len(fills)

            def emit_qk(f):
                ki, q0, base, n = fills[f]
                sc = psc.tile([P, PH], f32, tag="sc")
                diag = base == q0
                # QK segs first (only need kt/qt); the diag accumulate-matmul
                # follows so early fills don't gate on the dcomb DVE chain.
                for so in range(0, n, 512):
                    nn = min(512, n - so)
                    rows = slice(0, DH) if so == 0 else slice(DH, P)
                    nc.tensor.matmul(
                        sc[:, so : so + nn],
                        lhsT=kt[rows, q0 : q0 + P],
                        rhs=qt[rows, base + so : base + so + nn],
                        start=True,
                        stop=(not diag) if so == 0 else True,
                        skip_group_check=True,
                    )
                if diag:
                    nc.tensor.matmul(
                        sc[:, 0:P], lhsT=aui[:], rhs=dcomb3[:, ki, :],
                        start=False, stop=True, skip_group_check=True,
                    )
                return sc

            last_ki = fills[-1][0]

            def epi_chunk(c0, w):
                # transpose-back + divide + store for q columns [c0, c0+w)
                nq = w // P
                NP = PH // P
                nc.vector.tensor_copy(
                    outTs[0 : DH + 1, c0 : c0 + w], outT[:, c0 - lo : c0 - lo + w]
                )
                onat = outp.tile([P, NP * OW], f16, tag="onat", name="onat")
                onat3 = onat[:, 0 : nq * OW].rearrange("p (n e) -> p n e", e=OW)
                nc.sync.dma_start_transpose(out=onat3, in_=outTs[:, c0 : c0 + w])
                recip = outp.tile([P, NP], f32, tag="recip", name="recip")
                nc.vector.reciprocal(recip[:, 0:nq, None], onat3[:, :, DH : DH + 1])
                ofin = outp.tile([P, NP * DH], f32, tag="ofin", name="ofin")
                nc.vector.tensor_mul(
                    ofin[:, 0 : nq * DH].rearrange("p (n d) -> p n d", d=DH),
                    onat3[:, :, 0:DH],
                    recip[:, 0:nq, None].to_broadcast([P, nq, DH]),
                )
                nc.sync.dma_start(
                    out=out_d[h].rearrange("(n p) d -> p n d", p=P)[
                        :, c0 // P : c0 // P + nq, :
                    ],
                    in_=ofin[:, 0 : nq * DH].rearrange("p (n d) -> p n d", d=DH),
                )

            def emit_pv(f, slab):
                ki, q0, base, n = fills[f]
                for qb in range(base // 512, (base + n - 1) // 512 + 1):
                    g0 = max(base, qb * 512)
                    g1 = min(base + n, (qb + 1) * 512)
                    stop_f = min(last_ki, 4 * qb + 3)
                    nc.tensor.matmul(
                        outT[:, g0 - lo : g1 - lo],
                        lhsT=v3[:, ki, 0 : DH + 1],
                        rhs=slab[:, g0 - base : g1 - base],
                        start=(ki == 0),
                        stop=(ki == stop_f),
                        skip_group_check=True,
                    )
                    if split and ki == stop_f:
                        epi_chunk(qb * 512, 512)

            scs = {0: emit_qk(0)}
            if nf > 1:
                scs[1] = emit_qk(1)
            pend = []  # PV runs one fill behind its exp to lengthen the ring
            for f, (ki, q0, base, n) in enumerate(fills):
                sc = scs.pop(f)
                slab = slabp.tile([P, PH], f16, tag="slab")
                nc.scalar.activation(
                    slab[:, 0:n], sc[:, 0:n], AF.Exp, scale=INV_SCALE
                )
                if f + 2 < nf:
                    scs[f + 2] = emit_qk(f + 2)
                if pend:
                    emit_pv(*pend.pop(0))
                pend.append((f, slab))
            while pend:
                emit_pv(*pend.pop(0))
            if not split:
                epi_chunk(lo, PH)

    # ---- schedule: h1's loads/DVE-prep/xbar transposes are emitted early so
    # they fill idle engines during h0's main loop; only h0 uses PE transposes.
    # DVE-queue order matters (strict FIFO): h0 casts and transpose copies go
    # first; the rbc-dependent const math waits on its DMA where that's free.
    loads = [issue_loads(h) for h in range(HPC)]
    qf0, kf0 = prep_cast(0, *loads[0])
    qt0, kt0 = prep_tp_pe(qf0, kf0)
    emit_rb_consts()
    v30, dcomb30 = prep_dcomb(0, qf0, loads[0][2])
    junk_into(count=8, n=256)  # bridge the dcomb-wait gap, keep HAM at 8/8
    qf1, kf1 = prep_cast(1, *loads[1])
    v31, dcomb31 = prep_dcomb(1, qf1, loads[1][2])
    qt1, kt1 = prep_tp_xbar(qf1, kf1)
    outTs0 = outp.tile([OW, S], f16, tag="outTs")
    nc.gpsimd.memset(outTs0[DH:OW, :], 0.0)
    outTs1 = outp.tile([OW, S], f16, tag="outTs")
    nc.gpsimd.memset(outTs1[DH:OW, :], 0.0)
    main_loop(0, qt0, kt0, v30, dcomb30, outTs0, range(S // PH))
    main_loop(1, qt1, kt1, v31, dcomb31, outTs1, range(S // PH), split_last=True)


def build_nc(debug=False):
    from contextlib import ExitStack

    nc = bacc.Bacc("TRN2", target_bir_lowering=False, debug=debug, num_devices=N_CORES)
    q_d = nc.dram_tensor("query", [HPC, S, DH], f32, kind="ExternalInput").ap()
    k_d = nc.dram_tensor("key", [HPC, S, DH], f32, kind="ExternalInput").ap()
    v_d = nc.dram_tensor("value", [HPC, S, DH], f32, kind="ExternalInput").ap()
    rw_d = nc.dram_tensor("R_w", [3, DH], f32, kind="ExternalInput").ap()
    rb_d = nc.dram_tensor("R_b", [3], f32, kind="ExternalInput").ap()
    out_d = nc.dram_tensor("out", [HPC, S, DH], f32, kind="ExternalOutput").ap()
    with tile.TileContext(nc) as tc, ExitStack() as ctx:
        _emit(ctx, tc, q_d, k_d, v_d, rw_d, rb_d, out_d)
    nc.finalize()
    return nc


_NC_CACHE = {}


def _get_nc():
    if "nc" not in _NC_CACHE:
        _NC_CACHE["nc"] = build_nc()
    return _NC_CACHE["nc"]


def kernel(query, key, value, R_w, R_b, trace=False):
    query = np.ascontiguousarray(np.asarray(query, dtype=np.float32))
    key = np.ascontiguousarray(np.asarray(key, dtype=np.float32))
    value = np.ascontiguousarray(np.asarray(value, dtype=np.float32))
    R_w = np.ascontiguousarray(np.asarray(R_w, dtype=np.float32))
    R_b = np.ascontiguousarray(np.asarray(R_b, dtype=np.float32))

    nc = _get_nc()
    in_maps = [
        {
            "query": query[c * HPC : (c + 1) * HPC],
            "key": key[c * HPC : (c + 1) * HPC],
            "value": value[c * HPC : (c + 1) * HPC],
            "R_w": R_w,
            "R_b": R_b,
        }
        for c in range(N_CORES)
    ]
    res = run_bass_kernel_spmd(nc, in_maps, core_ids=list(range(N_CORES)), trace=trace)
    out = np.concatenate([res.results[c]["out"] for c in range(N_CORES)], axis=0)
    if trace:
        kernel.last_results = res
    return out.astype(np.float32, copy=False)



# revision 22
# speedup vs baseline: 1.1465x; 1.1465x over previous
"""Trainium2 Bass kernel for causal attention with relative-position bias.

Problem (hardcoded): B=16 heads, S=2048, Dh=64, fp32 I/O.
  dots = Q@K^T; bias pos=Q@R_w^T+R_b gathered by sign(j-i)+1; causal mask
  (-1e10 above diag); softmax(dots/sqrt(512)); out = probs@V.

Algebra: within row q the gathered bias is a constant pos0[q] for k<q and
pos1[q] at k==q (k>q masked). Softmax is invariant to per-row constants, so
only the diagonal needs the delta v[q] = Q[q].(R_w[1]-R_w[0]) + R_b[1]-R_b[0].
Logits are small (|z|<=~2.2) so exp runs without max subtraction.

Layout: scores computed transposed, S^T[k,q] (k on partitions):
  S^T = (K^T chunk).T @ Q^T      (lhsT=K^T[64,128], rhs=Q^T[64,ncols])
  out^T[d,q]+denominator row = [V|1].T @ exp(S^T)  (accumulated over chunks)

Diagonal tiles: one accumulate-matmul  A_ui^T @ dcomb_ki  is emitted before
the QK matmul, where A_ui[m,k]=[m<=k] (so the product is a column cumsum) and
dcomb[m,q] = v[q]*([m==q]-[m==q+1]) + (-V0*[m>q] + rbd*[m==q]).  The cumsum
turns this into v[q]*[k==q] - V0*(k-q)*[k>q] + rbd*[k>=q]: position-bias
delta on the diagonal, -V0*(k-q) above it (exp == 0 in fp16), nothing below.
The exp is then uniform and PV needs no diagonal special-casing.

Q^T/K^T: head 0 via PE-mode transposes at startup (PE idle, warms HAM),
head 1 via xbar-DMA transposes overlapped with head 0's main loop.  The
transposed strips are [128, S]: rows 0:64 hold K^T/Q^T, rows 64:128 a
gpsimd SBUF-to-SBUF duplicate, so the two 512-col QK chunks of a fill run
CONCURRENTLY in independent PE row groups (contraction is only 64) --
auto row-tiling from the operands' base partition.

Input loads ride the sync HWDGE ring so the scalar (ACT) engine queue
carries only the exp stream -- exp at 1 elem/lane/cycle @1.2GHz over the
causal region is the binding resource.

Sharding: 16 heads -> 8 NeuronCores, 2 heads/core, no communication.
"""

import os
import sys

if "/opt/trn_rl_repo" not in sys.path:
    sys.path.insert(0, "/opt/trn_rl_repo")

import numpy as np

import concourse.bacc as bacc
import concourse.mybir as mybir
import concourse.tile as tile
from concourse.bass_utils import run_bass_kernel_spmd
from concourse.masks import make_identity, make_lower_triangular, make_upper_triangular

B, S, DH = 16, 2048, 64
N_CORES = 8
HPC = B // N_CORES  # heads per core
P = 128
NT = S // P  # 16 q/k tiles per head
HT = NT // 2  # tiles per load half
VW = 66  # V row width in SBUF: 64 values + ones col + pad (66*2B keeps 4B align)
OW = 80  # out^T rows padded to xbar multiple of 16 (64 vals + denom + 15 pad)
PH = 1024  # q-phase width
INV_SCALE = float(1.0 / np.sqrt(np.float32(512.0)))
V0 = 1000.0  # per-step causal mask magnitude; exp((z-V0)/scale) == 0 in fp16

f16 = mybir.dt.float16
f32 = mybir.dt.float32


def _emit(ctx, tc, q_d, k_d, v_d, rw_d, rb_d, out_d):
    nc = tc.nc
    AF = mybir.ActivationFunctionType

    const = ctx.enter_context(tc.tile_pool(name="const", bufs=1))
    ld = ctx.enter_context(tc.tile_pool(name="ld", bufs=2))
    hp = ctx.enter_context(tc.tile_pool(name="hp", bufs=2))
    slabp = ctx.enter_context(tc.tile_pool(name="slab", bufs=4))
    outp = ctx.enter_context(tc.tile_pool(name="outp", bufs=2))
    psc = ctx.enter_context(tc.tile_pool(name="psc", bufs=3, space="PSUM"))
    pout = ctx.enter_context(tc.tile_pool(name="pout", bufs=1, space="PSUM"))

    # PE warm-up junk buffer; memset first so the warm matmuls start early
    junk = const.tile([P, 512], f16)
    nc.gpsimd.memset(junk[:], 0.0)

    # broadcast R_w rows 0+1 and R_b[0:2] to all partitions (0-step DMA reads)
    # rbc rides the scalar ring: on the sync ring its tiny broadcast packets
    # finish ~5us late (round-robined behind the 12 big input loads), and the
    # whole DVE const chain -> dcomb -> first exp waits on it.
    rbc = const.tile([P, 2 * DH + 2], f32)
    nc.scalar.dma_start(
        out=rbc[:, 0 : 2 * DH], in_=rw_d[0:2, :].flatten()[None, :].partition_broadcast(P)
    )
    nc.scalar.dma_start(
        out=rbc[:, 2 * DH : 2 * DH + 2], in_=rb_d[None, 0:2].partition_broadcast(P)
    )

    # constants ----------------------------------------------------------
    idm = const.tile([P, P], f16)  # fp16 identity for PE-mode transpose
    make_identity(nc, idm[:])
    aui = const.tile([P, P], f16)  # A_ui[m,k] = 1 for m<=k (cumsum matmul lhsT)
    make_upper_triangular(nc, aui[:], val=1.0, diag=True)
    # idp[m,j] = [m==j] - [m==j+1]: cumsum of idp*v gives diag(v)
    idp = const.tile([P, P], f16)
    make_identity(nc, idp[:])
    nc.gpsimd.affine_select(
        out=idp[:], in_=idp[:], compare_op=mybir.AluOpType.not_equal,
        fill=-1.0, base=-1, pattern=[[-1, P]], channel_multiplier=1,
    )
    # bneg[m,q] = -V0*[m>q] + rbd*[m==q]; cumsum gives the causal mask + rbd
    id01 = const.tile([P, P], mybir.dt.int8)
    make_identity(nc, id01[:])
    bneg = const.tile([P, P], f16)
    make_lower_triangular(nc, bneg[:], val=-V0, diag=False)

    rd16 = const.tile([P, DH], f16)  # R_w[1]-R_w[0], fp16, bcast on partitions
    rbraw = const.tile([P, 2], f32)  # col 0: R_b[1]-R_b[0] (raw, pre-scale)
    rbf16 = const.tile([P, 1], f16)

    def emit_rb_consts():
        # Emitted AFTER the first head's casts: these wait on the rbc DMAs,
        # and the DVE queue is strict FIFO — put them where the wait is free.
        nc.vector.tensor_sub(rd16[:], rbc[:, DH : 2 * DH], rbc[:, 0:DH])
        nc.vector.tensor_sub(
            rbraw[:, 0:1], rbc[:, 2 * DH + 1 : 2 * DH + 2], rbc[:, 2 * DH : 2 * DH + 1]
        )
        nc.vector.tensor_copy(rbf16[:], rbraw[:, 0:1])
        nc.vector.copy_predicated(bneg[:], id01[:], rbf16[:, 0:1].to_broadcast([P, P]))

    # PE warm-up: junk matmuls while DMAs load (HAM at 8/8 by the first QK).
    warm0 = psc.tile([P, PH], f32, tag="sc")
    for _ in range(6):
        nc.tensor.matmul(
            warm0[:, 0:512], lhsT=junk[:, 0:P], rhs=junk[:], start=True,
            stop=True, skip_group_check=True,
        )

    def junk_into(count=1, n=P, t=None):
        # junk matmuls keep the HAM clock gate at 8/8 across micro-gaps.
        # Targets are PSUM tiles whose data is about to be overwritten
        # (start=True only resets has_written bits; readers never see junk).
        if t is None:
            t = warm0
        for _ in range(count):
            nc.tensor.matmul(
                t[0:32, 0:n], lhsT=junk[:, 0:32], rhs=junk[:, 0:n], start=True,
                stop=True, skip_group_check=True,
            )

    # preload the exp table set at t=0 so ACT_TABLE_LOAD overlaps input DMA
    tl16 = const.tile([P, 8], f16)
    nc.scalar.activation(tl16[:], junk[:, 0:8], AF.Exp, scale=0.0)

    def issue_loads(h):
        # fp32 loads on the scalar HWDGE ring, halves interleaved k0,q0,v0,
        # k1,q1,v1 so phase-0 operands land first.
        q32 = ld.tile([P, NT * DH], f32, tag="q32")
        k32 = ld.tile([P, NT * DH], f32, tag="k32")
        v32 = ld.tile([P, NT * DH], f32, tag="v32")
        for src, dst in ((k_d, k32), (q_d, q32), (v_d, v32)):
            for hf in range(2):
                tsl = slice(hf * HT, (hf + 1) * HT)
                nc.sync.dma_start(
                    out=dst[:].rearrange("p (n d) -> p n d", d=DH)[:, tsl, :],
                    in_=src[h].rearrange("(n p) d -> p n d", p=P)[:, tsl, :],
                )
        return q32, k32, v32

    def prep_cast(h, q32, k32, v32):
        qf = hp.tile([P, NT * DH], f16, tag="qf")
        kf = hp.tile([P, NT * DH], f16, tag="kf")
        nc.vector.tensor_copy(kf[:], k32[:])
        nc.vector.tensor_copy(qf[:], q32[:])
        return qf, kf

    def prep_dcomb(h, qf, v32):
        vaug = hp.tile([P, NT * VW], f16, tag="vaug")
        v3 = vaug[:].rearrange("p (n e) -> p n e", e=VW)
        nc.vector.tensor_copy(
            v3[:, :, 0:DH], v32[:].rearrange("p (n d) -> p n d", d=DH)
        )
        nc.gpsimd.memset(v3[:, :, DH : DH + 1], 1.0)

        # dcomb strip: per k-tile, idp*v[q] + bneg (see module docstring)
        t2 = ld.tile([P, NT * DH], f16, tag="t2")
        t2_3 = t2[:].rearrange("p (n d) -> p n d", d=DH)
        vq = hp.tile([P, NT], f32, tag="vq")
        vq16 = hp.tile([P, NT], f16, tag="vq16")
        dcomb = hp.tile([P, NT * P], f16, tag="dcomb")
        dcomb3 = dcomb[:].rearrange("p (n j) -> p n j", j=P)
        qf3 = qf[:].rearrange("p (n d) -> p n d", d=DH)
        for hf in range(2):
            sl = slice(hf * HT, (hf + 1) * HT)
            nc.vector.tensor_mul(
                t2_3[:, sl, :], qf3[:, sl, :],
                rd16[:, None, :].to_broadcast([P, HT, DH]),
            )
            nc.vector.tensor_reduce(
                out=vq[:, sl], in_=t2_3[:, sl, :],
                axis=mybir.AxisListType.X, op=mybir.AluOpType.add,
            )
            nc.vector.tensor_copy(vq16[:, sl], vq[:, sl])
            nc.vector.tensor_mul(
                dcomb3[:, sl, :],
                idp[:, None, :].to_broadcast([P, HT, P]),
                vq16[:, sl, None].to_broadcast([P, HT, P]),
            )
            nc.vector.tensor_add(
                dcomb3[:, sl, :], dcomb3[:, sl, :],
                bneg[:, None, :].to_broadcast([P, HT, P]),
            )
        return v3, dcomb3

    def prep_tp_pe(qf, kf):
        """PE-mode transposes (startup: DMAs busy, PE idle).  The phase-0-
        critical first-half copies ride the idle scalar engine so the DVE
        queue reaches the dcomb chain sooner; second-half copies (only
        needed for phase 1) are deferred via the returned closures and run
        on DVE after the dcomb chain."""
        def pe_transpose_to(src, tag):
            dst = hp.tile([P, S], f16, tag=tag, name=tag)
            tp = psc.tile([P, PH], f32, tag="sc")
            tp16 = tp[:].bitcast(f16)  # [128, 2048] fp16 view
            s3 = src[:].rearrange("p (n d) -> p n d", d=DH)
            for t in range(NT):
                nc.tensor.transpose(
                    tp16[0:DH, t * P : (t + 1) * P], s3[:, t, :], idm[:]
                )
                if t % 3 == 1:
                    junk_into()
            nc.scalar.copy(dst[0:DH, 0 : S // 2], tp16[0:DH, 0 : S // 2])
            nc.gpsimd.dma_start(out=dst[DH:P, 0 : S // 2], in_=dst[0:DH, 0 : S // 2])

            def finish():
                nc.vector.tensor_copy(dst[0:DH, S // 2 : S], tp16[0:DH, S // 2 : S])
                nc.gpsimd.dma_start(
                    out=dst[DH:P, S // 2 : S], in_=dst[0:DH, S // 2 : S]
                )
            return dst, finish

        kt, fin_kt = pe_transpose_to(kf, "kt")
        qt, fin_qt = pe_transpose_to(qf, "qt")
        return qt, kt, fin_kt, fin_qt

    def prep_tp_xbar(qf, kf):
        """xbar-DMA transposes (mid-flight: PE busy, DMA rings quiet).
        Folds on the sync ring, unfolds on the gpsimd (SWDGE) ring."""
        def xbar_transpose_to(src, tag):
            dst = hp.tile([P, S], f16, tag=tag, name=tag)
            fold = ld.tile([P, 8 * P], f16, tag="fold" + tag)
            nc.sync.dma_start_transpose(
                out=fold[:].rearrange("p (m r) -> p m r", r=P), in_=src[:]
            )
            d4 = dst[0:DH, :].rearrange("d (m j r) -> d m j r", j=2, r=P)
            f3 = fold[:].rearrange("p (m r) -> p m r", r=P)
            nc.gpsimd.dma_start(out=d4[:, :, 0, :], in_=f3[0:DH])
            nc.gpsimd.dma_start(out=d4[:, :, 1, :], in_=f3[DH:P])
            nc.gpsimd.dma_start(out=dst[DH:P, 0 : S // 2], in_=dst[0:DH, 0 : S // 2])
            nc.gpsimd.dma_start(out=dst[DH:P, S // 2 : S], in_=dst[0:DH, S // 2 : S])
            return dst

        kt = xbar_transpose_to(kf, "kt")
        qt = xbar_transpose_to(qf, "qt")
        return qt, kt

    def main_loop(h, qt, kt, v3, dcomb3, outTs, phases, split_last=False):
        for ph in phases:
            lo, hi = ph * PH, (ph + 1) * PH
            split = split_last and ph == phases[-1]
            fills = []
            for ki in range(NT):
                q0 = P * ki
                base = max(q0, lo)
                if base < hi:
                    fills.append((ki, q0, base, hi - base))
            outT = pout.tile([DH + 1, PH], f32, tag="outT")
            nf = len(fills)

            def emit_qk(f):
                ki, q0, base, n = fills[f]
                sc = psc.tile([P, PH], f32, tag="sc")
                diag = base == q0
                # QK segs first (only need kt/qt); the diag accumulate-matmul
                # follows so early fills don't gate on the dcomb DVE chain.
                for so in range(0, n, 512):
                    nn = min(512, n - so)
                    rows = slice(0, DH) if so == 0 else slice(DH, P)
                    nc.tensor.matmul(
                        sc[:, so : so + nn],
                        lhsT=kt[rows, q0 : q0 + P],
                        rhs=qt[rows, base + so : base + so + nn],
                        start=True,
                        stop=(not diag) if so == 0 else True,
                        skip_group_check=True,
                    )
                if diag:
                    nc.tensor.matmul(
                        sc[:, 0:P], lhsT=aui[:], rhs=dcomb3[:, ki, :],
                        start=False, stop=True, skip_group_check=True,
                    )
                return sc

            last_ki = fills[-1][0]

            def epi_chunk(c0, w):
                # transpose-back + divide + store for q columns [c0, c0+w)
                nq = w // P
                NP = PH // P
                nc.vector.tensor_copy(
                    outTs[0 : DH + 1, c0 : c0 + w], outT[:, c0 - lo : c0 - lo + w]
                )
                onat = outp.tile([P, NP * OW], f16, tag="onat", name="onat")
                onat3 = onat[:, 0 : nq * OW].rearrange("p (n e) -> p n e", e=OW)
                nc.sync.dma_start_transpose(out=onat3, in_=outTs[:, c0 : c0 + w])
                recip = outp.tile([P, NP], f32, tag="recip", name="recip")
                nc.vector.reciprocal(recip[:, 0:nq, None], onat3[:, :, DH : DH + 1])
                ofin = outp.tile([P, NP * DH], f32, tag="ofin", name="ofin")
                nc.vector.tensor_mul(
                    ofin[:, 0 : nq * DH].rearrange("p (n d) -> p n d", d=DH),
                    onat3[:, :, 0:DH],
                    recip[:, 0:nq, None].to_broadcast([P, nq, DH]),
                )
                nc.sync.dma_start(
                    out=out_d[h].rearrange("(n p) d -> p n d", p=P)[
                        :, c0 // P : c0 // P + nq, :
                    ],
                    in_=ofin[:, 0 : nq * DH].rearrange("p (n d) -> p n d", d=DH),
                )

            def emit_pv(f, slab):
                ki, q0, base, n = fills[f]
                for qb in range(base // 512, (base + n - 1) // 512 + 1):
                    g0 = max(base, qb * 512)
                    g1 = min(base + n, (qb + 1) * 512)
                    stop_f = min(last_ki, 4 * qb + 3)
                    nc.tensor.matmul(
                        outT[:, g0 - lo : g1 - lo],
                        lhsT=v3[:, ki, 0 : DH + 1],
                        rhs=slab[:, g0 - base : g1 - base],
                        start=(ki == 0),
                        stop=(ki == stop_f),
                        skip_group_check=True,
                    )
                    if split and ki == stop_f:
                        epi_chunk(qb * 512, 512)

            scs = {0: emit_qk(0)}
            if nf > 1:
                scs[1] = emit_qk(1)
            pend = []  # PV runs one fill behind its exp to lengthen the ring
            for f, (ki, q0, base, n) in enumerate(fills):
                sc = scs.pop(f)
                slab = slabp.tile([P, PH], f16, tag="slab")
                nc.scalar.activation(
                    slab[:, 0:n], sc[:, 0:n], AF.Exp, scale=INV_SCALE
                )
                if f + 2 < nf:
                    scs[f + 2] = emit_qk(f + 2)
                if pend:
                    emit_pv(*pend.pop(0))
                pend.append((f, slab))
            while pend:
                emit_pv(*pend.pop(0))
            if not split:
                epi_chunk(lo, PH)

    # ---- schedule: h1's loads/DVE-prep/xbar transposes are emitted early so
    # they fill idle engines during h0's main loop; only h0 uses PE transposes.
    # DVE-queue order matters (strict FIFO): h0 casts and transpose copies go
    # first; the rbc-dependent const math waits on its DMA where that's free.
    loads = [issue_loads(h) for h in range(HPC)]
    qf0, kf0 = prep_cast(0, *loads[0])
    qt0, kt0, fin_kt0, fin_qt0 = prep_tp_pe(qf0, kf0)
    emit_rb_consts()
    v30, dcomb30 = prep_dcomb(0, qf0, loads[0][2])
    fin_qt0()  # second-half copies: DVE, after the dcomb chain
    fin_kt0()
    junk_into(count=8, n=256)  # bridge the dcomb-wait gap, keep HAM at 8/8
    qf1, kf1 = prep_cast(1, *loads[1])
    v31, dcomb31 = prep_dcomb(1, qf1, loads[1][2])
    qt1, kt1 = prep_tp_xbar(qf1, kf1)
    outTs0 = outp.tile([OW, S], f16, tag="outTs")
    nc.gpsimd.memset(outTs0[DH:OW, :], 0.0)
    outTs1 = outp.tile([OW, S], f16, tag="outTs")
    nc.gpsimd.memset(outTs1[DH:OW, :], 0.0)
    main_loop(0, qt0, kt0, v30, dcomb30, outTs0, range(S // PH))
    main_loop(1, qt1, kt1, v31, dcomb31, outTs1, range(S // PH), split_last=True)


def build_nc(debug=False):
    from contextlib import ExitStack

    nc = bacc.Bacc("TRN2", target_bir_lowering=False, debug=debug, num_devices=N_CORES)
    q_d = nc.dram_tensor("query", [HPC, S, DH], f32, kind="ExternalInput").ap()
    k_d = nc.dram_tensor("key", [HPC, S, DH], f32, kind="ExternalInput").ap()
    v_d = nc.dram_tensor("value", [HPC, S, DH], f32, kind="ExternalInput").ap()
    rw_d = nc.dram_tensor("R_w", [3, DH], f32, kind="ExternalInput").ap()
    rb_d = nc.dram_tensor("R_b", [3], f32, kind="ExternalInput").ap()
    out_d = nc.dram_tensor("out", [HPC, S, DH], f32, kind="ExternalOutput").ap()
    with tile.TileContext(nc) as tc, ExitStack() as ctx:
        _emit(ctx, tc, q_d, k_d, v_d, rw_d, rb_d, out_d)
    nc.finalize()
    return nc


_NC_CACHE = {}


def _get_nc():
    if "nc" not in _NC_CACHE:
        _NC_CACHE["nc"] = build_nc()
    return _NC_CACHE["nc"]


def kernel(query, key, value, R_w, R_b, trace=False):
    query = np.ascontiguousarray(np.asarray(query, dtype=np.float32))
    key = np.ascontiguousarray(np.asarray(key, dtype=np.float32))
    value = np.ascontiguousarray(np.asarray(value, dtype=np.float32))
    R_w = np.ascontiguousarray(np.asarray(R_w, dtype=np.float32))
    R_b = np.ascontiguousarray(np.asarray(R_b, dtype=np.float32))

    nc = _get_nc()
    in_maps = [
        {
            "query": query[c * HPC : (c + 1) * HPC],
            "key": key[c * HPC : (c + 1) * HPC],
            "value": value[c * HPC : (c + 1) * HPC],
            "R_w": R_w,
            "R_b": R_b,
        }
        for c in range(N_CORES)
    ]
    res = run_bass_kernel_spmd(nc, in_maps, core_ids=list(range(N_CORES)), trace=trace)
    out = np.concatenate([res.results[c]["out"] for c in range(N_CORES)], axis=0)
    if trace:
        kernel.last_results = res
    return out.astype(np.float32, copy=False)

